# revision 8
# baseline (speedup 1.0000x reference)
"""Trainium2 Bass kernel for nn_CoSemiGNN (GATv2 + MA block + EvolveGCNO GCN + head).

kernel(**inputs) takes the FULL unsharded inputs (numpy) and returns the FULL
output tuple (out_line [N], out [N, DIM3]) matching reference().

Sharding: nodes sharded across 8 NeuronCores by destination; edges are routed
to the core owning their dst and packed (host side) into 128-slot columns so
that no dst's edge segment spans a column. Per-edge source rows are fetched
with dma_gather; segment softmax sums and GCN aggregation are computed with
PE one-hot matmuls accumulating in PSUM (scatter-free). The only collective
is one AllGather (xw).
"""
import numpy as np
from contextlib import ExitStack

from concourse import bacc, bass, mybir, tile
from concourse.bass_utils import run_bass_kernel_spmd
from concourse.masks import make_identity

F = mybir.dt.float32
I16 = mybir.dt.int16
AF = mybir.ActivationFunctionType
OP = mybir.AluOpType
AX = mybir.AxisListType

N, FIN, DIM, DIM2, DIM3, H = 32768, 256, 128, 256, 128, 4
HD = H * DIM
NCORES = 8
P = 128

_BUILD_CACHE = {}


class _SliceAP:
    """Adapter so a sliced AP can be used like a tile (supports [key])."""

    def __init__(self, ap):
        self._ap = ap

    def __getitem__(self, key):
        if key == slice(None):
            return self._ap
        return self._ap[key]


# --------------------------------------------------------------------------
# host-side graph packing
# --------------------------------------------------------------------------

def pack_graph(src, dst, n_cores, nloc, nn):
    ranges = nloc // P
    deg = np.bincount(dst, minlength=nn).astype(np.float64)
    dinv = 1.0 / np.sqrt(np.maximum(deg, 1e-12))
    norm_all = (dinv[src] * dinv[dst]).astype(np.float32)

    core_of = dst // nloc
    percore_raw = []
    maxcols = 0
    for c in range(n_cores):
        m = core_of == c
        s_c, d_c, n_c = src[m], dst[m] - c * nloc, norm_all[m]
        order = np.argsort(d_c, kind="stable")
        s_c, d_c, n_c = s_c[order], d_c[order], n_c[order]
        cols_meta = []
        for r in range(ranges):
            rm = (d_c // P) == r
            sr, dr, nr = s_c[rm], d_c[rm] - r * P, n_c[rm]
            bounds = np.flatnonzero(np.diff(dr)) + 1
            segs = np.split(np.arange(len(dr)), bounds)
            cur = []
            ncols_r = 0
            for seg in segs:
                if len(seg) == 0:
                    continue
                assert len(seg) <= P, "segment larger than a column"
                if len(cur) + len(seg) > P:
                    cols_meta.append((r, cur))
                    ncols_r += 1
                    cur = []
                cur.extend((sr[i], dr[i], nr[i]) for i in seg)
            cols_meta.append((r, cur))
            ncols_r += 1
            maxcols = max(maxcols, ncols_r)
        percore_raw.append(cols_meta)

    CPR = maxcols
    COLS = ranges * CPR
    per_core = []
    for c in range(n_cores):
        srcidx = np.zeros(COLS * P, np.int64)
        A = np.zeros((COLS, P, P), np.float32)
        normv = np.zeros((P, COLS), np.float32)
        next_col = np.zeros(ranges, np.int64)
        for r, entries in percore_raw[c]:
            col = r * CPR + next_col[r]
            next_col[r] += 1
            for slot, (sv, lane, nv) in enumerate(entries):
                srcidx[col * P + slot] = sv
                A[col, slot, lane] = 1.0
                normv[slot, col] = nv
        w = srcidx.astype(np.int16).reshape(COLS * P // 16, 16).T
        w = np.tile(w, (8, 1))
        per_core.append({
            "srcidx": np.ascontiguousarray(w),
            "A": A,
            "AT": np.ascontiguousarray(A.transpose(0, 2, 1)),
            "normv": normv,
        })
    return CPR, per_core


def prep_weights(inp):
    f32 = lambda x: np.ascontiguousarray(np.asarray(x, np.float32))

    def col(b, parts):
        return np.ascontiguousarray(np.asarray(b, np.float32).reshape(parts, P).T)

    w = {}
    w["glw_t"] = f32(inp["gl_w"]).T.copy()
    w["grw_t"] = f32(inp["gr_w"]).T.copy()
    w["glb_row"] = f32(inp["gl_b"]).reshape(1, HD)
    w["grb_row"] = f32(inp["gr_b"]).reshape(1, HD)
    w["att_row"] = f32(inp["gat_att"]).reshape(1, HD)
    w["gat_b"] = col(inp["gat_b"], 1)
    w["emb1w_t"] = f32(inp["emb1_w"]).T.copy()
    w["emb1_b"] = col(inp["emb1_b"], 1)
    w["ma1w_t"] = f32(inp["ma1_w"]).T.copy()
    w["ma1_b"] = col(inp["ma1_b"], 1)
    w["ma2w_t"] = f32(inp["ma2_w"]).T.copy()
    w["ma2_b"] = col(inp["ma2_b"], 2)
    w["lvw_t"] = f32(inp["lv_w"]).T.copy()
    w["lv_b"] = col(inp["lv_b"], 2)
    w["wvw_t"] = f32(inp["inproj_w"])[2 * DIM2:].T.copy()
    w["wv_b"] = col(np.asarray(inp["inproj_b"])[2 * DIM2:], 2)
    w["outw_t"] = f32(inp["outproj_w"]).T.copy()
    w["out_b"] = col(inp["outproj_b"], 2)
    w["maln_g"] = col(inp["ma_ln_g"], 2)
    w["maln_b"] = col(inp["ma_ln_b"], 2)
    w["fn1_g"] = col(inp["fn1_g"], 2)
    w["fn1_b"] = col(inp["fn1_b"], 2)
    egw = f32(inp["eg_W"])
    w["egw_t"] = egw.T.copy()
    w["egw_row"] = egw.copy()
    wih, whh = f32(inp["gru_wih"]), f32(inp["gru_whh"])
    bih, bhh = f32(inp["gru_bih"]), f32(inp["gru_bhh"])
    w["wrz_t"] = (wih[: 2 * DIM2] + whh[: 2 * DIM2]).T.copy()
    w["brz_row"] = (bih[: 2 * DIM2] + bhh[: 2 * DIM2]).reshape(1, 2 * DIM2)
    w["win_t"] = wih[2 * DIM2:].T.copy()
    w["bin_row"] = bih[2 * DIM2:].reshape(1, DIM2)
    w["whn_t"] = whh[2 * DIM2:].T.copy()
    w["bhn_row"] = bhh[2 * DIM2:].reshape(1, DIM2)
    w["nev_g"] = col(inp["nev_g"], 2)
    w["nev_b"] = col(inp["nev_b"], 2)
    w["fc1w_t"] = f32(inp["fc1_w"]).T.copy()
    w["fc1_b"] = col(inp["fc1_b"], 1)
    w["fc2w_t"] = f32(inp["fc2_w"]).T.copy()
    w["fc2_b"] = col(inp["fc2_b"], 1)
    w["ng_g"] = col(inp["ng_g"], 1)
    w["ng_b"] = col(inp["ng_b"], 1)
    w["clsw_col"] = f32(inp["cls_w"]).reshape(DIM3, 1).copy()
    w["cls_b"] = f32(inp["cls_b"]).reshape(1, 1)
    return w


WEIGHT_SHAPES = {
    "glw_t": [FIN, HD], "grw_t": [FIN, HD], "glb_row": [1, HD],
    "grb_row": [1, HD], "att_row": [1, HD], "gat_b": [P, 1],
    "emb1w_t": [FIN, DIM], "emb1_b": [P, 1],
    "ma1w_t": [2 * DIM, DIM], "ma1_b": [P, 1],
    "ma2w_t": [DIM, DIM2], "ma2_b": [P, 2],
    "lvw_t": [DIM2, DIM2], "lv_b": [P, 2],
    "wvw_t": [DIM2, DIM2], "wv_b": [P, 2],
    "outw_t": [DIM2, DIM2], "out_b": [P, 2],
    "maln_g": [P, 2], "maln_b": [P, 2], "fn1_g": [P, 2], "fn1_b": [P, 2],
    "egw_t": [DIM2, DIM2], "egw_row": [DIM2, DIM2],
    "wrz_t": [DIM2, 2 * DIM2], "brz_row": [1, 2 * DIM2],
    "win_t": [DIM2, DIM2], "bin_row": [1, DIM2],
    "whn_t": [DIM2, DIM2], "bhn_row": [1, DIM2],
    "nev_g": [P, 2], "nev_b": [P, 2],
    "fc1w_t": [DIM2, DIM3], "fc1_b": [P, 1],
    "fc2w_t": [DIM3, DIM3], "fc2_b": [P, 1],
    "ng_g": [P, 1], "ng_b": [P, 1],
    "clsw_col": [DIM3, 1], "cls_b": [1, 1],
}


# --------------------------------------------------------------------------
# device kernel builder
# --------------------------------------------------------------------------

def build_bass(n_cores, nn, nloc, CPR, CC=2, CH_T=1024):
    ranges = nloc // P
    COLS = ranges * CPR
    assert COLS % CC == 0
    NBLK = nn // P
    LBLK = nloc // P
    TCH = nloc // CH_T

    nc = bacc.Bacc("TRN2", target_bir_lowering=False, debug=False,
                   num_devices=n_cores)

    featT_d = nc.dram_tensor("featT", [FIN, nn], F, kind="ExternalInput")
    featTloc_d = nc.dram_tensor("featTloc", [FIN, nloc], F, kind="ExternalInput")
    srcidx_d = nc.dram_tensor("srcidx", [P, 8 * COLS], I16, kind="ExternalInput")
    A_d = nc.dram_tensor("A", [COLS, P, P], F, kind="ExternalInput")
    AT_d = nc.dram_tensor("AT", [COLS, P, P], F, kind="ExternalInput")
    normv_d = nc.dram_tensor("normv", [P, COLS], F, kind="ExternalInput")
    wd = {k: nc.dram_tensor(k, sh, F, kind="ExternalInput")
          for k, sh in WEIGHT_SHAPES.items()}

    out_mat_d = nc.dram_tensor("out_mat", [nloc, DIM3], F, kind="ExternalOutput")
    out_line_d = nc.dram_tensor("out_line", [1, nloc], F, kind="ExternalOutput")

    rep_id = [0]

    with tile.TileContext(nc) as tc, ExitStack() as ctx:
        dram = ctx.enter_context(tc.tile_pool(name="dram", bufs=1, space="DRAM"))
        xl_full = dram.tile([nn, HD], F)
        xr_loc = dram.tile([nloc, HD], F)
        xw_loc = dram.tile([nloc, DIM2], F)
        xw_full = dram.tile([nn, DIM2], F)

        const = ctx.enter_context(tc.tile_pool(name="const", bufs=1))
        ident = const.tile([P, P], F)
        make_identity(nc, ident[:])
        ones_row = const.tile([1, P], F)
        nc.vector.memset(ones_row[:], 1.0)
        invn256 = const.tile([P, 1], F)
        nc.vector.memset(invn256[:], 1.0 / DIM2)
        invn128 = const.tile([P, 1], F)
        nc.vector.memset(invn128[:], 1.0 / DIM3)
        eps_t = const.tile([P, 1], F)
        nc.vector.memset(eps_t[:], 1e-12)

        wt = {}
        wpool = ctx.enter_context(tc.tile_pool(name="w", bufs=1))
        for k, sh in WEIGHT_SHAPES.items():
            if sh[0] <= P:
                t = wpool.tile(sh, F, tag=k)
                nc.sync.dma_start(t[:], wd[k][:])
                wt[k] = [t]
            else:
                ts = []
                for kt in range(sh[0] // P):
                    t = wpool.tile([P, sh[1]], F, tag=f"{k}{kt}")
                    nc.sync.dma_start(t[:], wd[k][kt * P:(kt + 1) * P, :])
                    ts.append(t)
                wt[k] = ts

        def replicate_row(row_tile, width, tag, pool):
            """Broadcast a [1, width] row across all 128 partitions."""
            rep_id[0] += 1
            with tc.tile_pool(name=f"repps{rep_id[0]}", bufs=1,
                              space="PSUM") as rp:
                ps = rp.tile([P, width], F)
                nc.tensor.matmul(ps[:], lhsT=ones_row[:], rhs=row_tile[:, :width],
                                 start=True, stop=True)
                sb = pool.tile([P, width], F, tag=tag)
                nc.scalar.copy(sb[:], ps[:])
            return sb

        actp = ctx.enter_context(tc.tile_pool(name="actp", bufs=1))
        xg_fm = actp.tile([P, nloc], F, tag="xg")
        x0_fm = actp.tile([P, nloc], F, tag="x0")

        # ==================================================================
        # Phase 1: xl_full (all nodes), xr_loc + x0 (own shard)
        # ==================================================================
        with tc.tile_pool(name="p1", bufs=3) as p1, \
             tc.tile_pool(name="p1c", bufs=1) as p1c, \
             tc.tile_pool(name="p1ps", bufs=2, space="PSUM") as p1ps:
            glb_rep = replicate_row(wt["glb_row"][0], HD, "glbr", p1c)
            grb_rep = replicate_row(wt["grb_row"][0], HD, "grbr", p1c)
            ftloc = [p1c.tile([P, nloc], F, tag=f"ftloc{kt}", name=f"ftloc{kt}") for kt in range(2)]
            for kt in range(2):
                nc.sync.dma_start(ftloc[kt][:], featTloc_d[kt * P:(kt + 1) * P, :])
            for blk in range(NBLK):
                sl = slice(blk * P, (blk + 1) * P)
                fts = []
                for kt in range(2):
                    t = p1.tile([P, P], F, tag=f"ft{kt}")
                    nc.sync.dma_start(t[:], featT_d[kt * P:(kt + 1) * P, sl])
                    fts.append(t)
                ps = p1ps.tile([P, HD], F, tag="xlps")
                for kt in range(2):
                    nc.tensor.matmul(ps[:], lhsT=fts[kt][:], rhs=wt["glw_t"][kt][:],
                                     start=(kt == 0), stop=(kt == 1))
                xsb = p1.tile([P, HD], F, tag="xlsb")
                nc.vector.tensor_add(xsb[:], ps[:], glb_rep[:])
                nc.sync.dma_start(xl_full[sl, :], xsb[:])
            for blk in range(LBLK):
                sl = slice(blk * P, (blk + 1) * P)
                ps = p1ps.tile([P, HD], F, tag="xrps")
                for kt in range(2):
                    nc.tensor.matmul(ps[:], lhsT=ftloc[kt][:, sl],
                                     rhs=wt["grw_t"][kt][:],
                                     start=(kt == 0), stop=(kt == 1))
                xsb = p1.tile([P, HD], F, tag="xrsb")
                nc.vector.tensor_add(xsb[:], ps[:], grb_rep[:])
                nc.sync.dma_start(xr_loc[sl, :], xsb[:])
            for nchk in range(nloc // 512):
                sl = slice(nchk * 512, (nchk + 1) * 512)
                ps = p1ps.tile([P, 512], F, tag="x0ps")
                for kt in range(2):
                    nc.tensor.matmul(ps[:], lhsT=wt["emb1w_t"][kt][:],
                                     rhs=ftloc[kt][:, sl],
                                     start=(kt == 0), stop=(kt == 1))
                nc.scalar.activation(x0_fm[:, sl], ps[:], AF.Identity,
                                     bias=wt["emb1_b"][0][:, :1])

        # ==================================================================
        # Phase 2: GAT edge loop, fused per-range post -> xg_fm
        # ==================================================================
        with tc.tile_pool(name="e2c", bufs=1) as e2c:
            att_rep = replicate_row(wt["att_row"][0], HD, "attr", e2c)
            idx_t = e2c.tile([P, 8 * COLS], I16, tag="idx")
            nc.sync.dma_start(idx_t[:], srcidx_d[:])
            with tc.tile_pool(name="e2", bufs=3) as e2, \
                 tc.tile_pool(name="e2a", bufs=2 * CC) as e2a, \
                 tc.tile_pool(name="e2xrb", bufs=2) as e2xrb, \
                 tc.tile_pool(name="e2post", bufs=2) as e2post, \
                 tc.tile_pool(name="xrps", bufs=2, space="PSUM") as xrps, \
                 tc.tile_pool(name="aggps", bufs=2, space="PSUM") as aggps, \
                 tc.tile_pool(name="denps", bufs=1, space="PSUM") as denps, \
                 tc.tile_pool(name="trps", bufs=1, space="PSUM") as trps:
                xrb, agg_ps, den_ps = None, None, None
                for ch in range(COLS // CC):
                    cols = [ch * CC + i for i in range(CC)]
                    xlg = e2.tile([P, CC, HD], F, tag="xlg")
                    nc.gpsimd.dma_gather(
                        xlg[:], xl_full[:],
                        idx_t[:, ch * CC * 8:(ch + 1) * CC * 8],
                        CC * P, CC * P, HD, single_packet=(CC * P <= 1024))
                    xr_ps = xrps.tile([P, CC, HD], F, tag="xrp")
                    A_sb = []
                    for i, col in enumerate(cols):
                        r = col // CPR
                        a_t = e2a.tile([P, P], F, tag="Asb")
                        nc.sync.dma_start(a_t[:], A_d[col, :, :])
                        A_sb.append(a_t)
                        at_t = e2a.tile([P, P], F, tag="ATsb")
                        nc.sync.dma_start(at_t[:], AT_d[col, :, :])
                        if col % CPR == 0:
                            xrb = e2xrb.tile([P, HD], F, tag="xrb")
                            nc.sync.dma_start(xrb[:], xr_loc[r * P:(r + 1) * P, :])
                        nc.tensor.matmul(xr_ps[:, i, :], lhsT=at_t[:], rhs=xrb[:],
                                         start=True, stop=True)
                    t_t = e2.tile([P, CC, HD], F, tag="tt")
                    nc.vector.tensor_add(t_t[:], xlg[:], xr_ps[:])
                    nc.vector.scalar_tensor_tensor(t_t[:], t_t[:], 0.2, t_t[:],
                                                   OP.mult, OP.max)
                    nc.vector.tensor_mul(
                        t_t[:], t_t[:],
                        att_rep[:].unsqueeze(1).to_broadcast([P, CC, HD]))
                    ex = e2.tile([P, CC, H], F, tag="ex")
                    nc.vector.tensor_reduce(
                        ex[:], t_t[:].rearrange("p c (h d) -> p c h d", h=H),
                        AX.X, OP.add)
                    nc.scalar.activation(ex[:], ex[:], AF.Exp)
                    nc.vector.tensor_mul(
                        xlg[:].rearrange("p c (h d) -> p c h d", h=H),
                        xlg[:].rearrange("p c (h d) -> p c h d", h=H),
                        ex[:].unsqueeze(3).to_broadcast([P, CC, H, DIM]))
                    for i, col in enumerate(cols):
                        r = col // CPR
                        first = col % CPR == 0
                        last = col % CPR == CPR - 1
                        if first:
                            agg_ps = aggps.tile([P, HD], F, tag="aggp")
                            den_ps = denps.tile([P, H], F, tag="denp")
                        nc.tensor.matmul(agg_ps[:], lhsT=A_sb[i][:],
                                         rhs=xlg[:, i, :],
                                         start=first, stop=last)
                        nc.tensor.matmul(den_ps[:], lhsT=A_sb[i][:],
                                         rhs=ex[:, i, :],
                                         start=first, stop=last)
                        if last:
                            rec = e2post.tile([P, H], F, tag="rec")
                            nc.vector.reciprocal(rec[:], den_ps[:])
                            nc.vector.tensor_scalar_mul(rec[:], rec[:], 1.0 / H)
                            agg_sb = e2post.tile([P, HD], F, tag="aggsb")
                            nc.scalar.copy(agg_sb[:], agg_ps[:])
                            a4 = agg_sb[:].rearrange("p (h d) -> p h d", h=H)
                            gat_sb = e2post.tile([P, DIM], F, tag="gatsb")
                            gtmp = e2post.tile([P, DIM], F, tag="gtmp")
                            for h in range(H):
                                dt = gat_sb if h == 0 else gtmp
                                nc.vector.tensor_mul(
                                    dt[:], a4[:, h, :],
                                    rec[:, h:h + 1].to_broadcast([P, DIM]))
                                if h > 0:
                                    nc.vector.tensor_add(gat_sb[:], gat_sb[:],
                                                         gtmp[:])
                            tr = trps.tile([P, P], F, tag="gtr")
                            nc.tensor.transpose(tr[:], gat_sb[:], ident[:])
                            nc.scalar.activation(
                                xg_fm[:, r * P:(r + 1) * P], tr[:], AF.Lrelu,
                                bias=wt["gat_b"][0][:, :1])

        # ==================================================================
        # dense helpers (used with whatever pools are current)
        # ==================================================================
        dn = {}

        def mm_fm(in_tiles, width, wname, bias_name=None, act_f=None,
                  out_tag="fmtmp"):
            wts = wt[wname]
            mts = wts[0].shape[1] // P
            outs = [dn["pool"].tile([P, width], F, tag=f"{out_tag}{mt}",
                                    name=f"{out_tag}{mt}")
                    for mt in range(mts)]
            for mt in range(mts):
                for j in range(width // 512):
                    sl = slice(j * 512, (j + 1) * 512)
                    ps = dn["ps"].tile([P, 512], F, tag="mmps")
                    for kt in range(len(in_tiles)):
                        nc.tensor.matmul(ps[:],
                                         lhsT=wts[kt][:, mt * P:(mt + 1) * P],
                                         rhs=in_tiles[kt][:, sl],
                                         start=(kt == 0),
                                         stop=(kt == len(in_tiles) - 1))
                    b = wt[bias_name][0][:, mt:mt + 1] if bias_name else 0.0
                    nc.scalar.activation(outs[mt][:, sl], ps[:],
                                         act_f or AF.Identity, bias=b)
            return outs

        def layer_norm(in_tiles, width, gname, bname, invn_tile,
                       out_tiles=None, out_tag="lnout"):
            nk = len(in_tiles)
            mrow = dn["ln"].tile([1, width], F, tag="mrow")
            qrow = dn["ln"].tile([1, width], F, tag="qrow")
            sqs = [dn["ln"].tile([P, width], F, tag=f"lnsq{kt}",
                                 name=f"lnsq{kt}")
                   for kt in range(nk)]
            for kt in range(nk):
                nc.scalar.square(sqs[kt][:], in_tiles[kt][:, :width])
            for j in range(width // 512):
                sl = slice(j * 512, (j + 1) * 512)
                psm = dn["rowps"].tile([1, 512], F, tag="prow")
                for kt in range(nk):
                    nc.tensor.matmul(psm[:], lhsT=invn_tile[:, :1],
                                     rhs=in_tiles[kt][:, sl],
                                     start=(kt == 0), stop=(kt == nk - 1))
                nc.vector.tensor_copy(mrow[:, sl], psm[:])
                psq = dn["rowps"].tile([1, 512], F, tag="prow")
                for kt in range(nk):
                    nc.tensor.matmul(psq[:], lhsT=invn_tile[:, :1],
                                     rhs=sqs[kt][:, sl],
                                     start=(kt == 0), stop=(kt == nk - 1))
                nc.vector.tensor_copy(qrow[:, sl], psq[:])
            vrow = dn["ln"].tile([1, width], F, tag="vrow")
            nc.vector.tensor_mul(vrow[:], mrow[:], mrow[:])
            nc.vector.tensor_sub(vrow[:], qrow[:], vrow[:])
            nc.scalar.activation(vrow[:], vrow[:], AF.Sqrt, bias=eps_t[:1, :1])
            arow = dn["ln"].tile([1, width], F, tag="arow")
            nc.vector.reciprocal(arow[:], vrow[:])
            crow = dn["ln"].tile([1, width], F, tag="crow")
            nc.vector.scalar_tensor_tensor(crow[:], mrow[:], -1.0, arow[:],
                                           OP.mult, OP.mult)
            if out_tiles is None:
                out_tiles = [dn["pool"].tile([P, width], F, tag=f"{out_tag}{kt}",
                                             name=f"{out_tag}{kt}")
                             for kt in range(nk)]
            for j in range(width // 512):
                sl = slice(j * 512, (j + 1) * 512)
                psa = dn["rowps"].tile([P, 512], F, tag="pbc")
                nc.tensor.matmul(psa[:], lhsT=ones_row[:], rhs=arow[:, sl],
                                 start=True, stop=True)
                for kt in range(nk):
                    nc.vector.tensor_mul(out_tiles[kt][:, sl],
                                         in_tiles[kt][:, sl], psa[:])
                psc = dn["rowps"].tile([P, 512], F, tag="pbc")
                nc.tensor.matmul(psc[:], lhsT=ones_row[:], rhs=crow[:, sl],
                                 start=True, stop=True)
                for kt in range(nk):
                    o = out_tiles[kt]
                    nc.vector.tensor_add(o[:, sl], o[:, sl], psc[:])
                    nc.vector.tensor_scalar(o[:, sl], o[:, sl],
                                            wt[gname][0][:, kt:kt + 1],
                                            wt[bname][0][:, kt:kt + 1],
                                            OP.mult, OP.add)
            return out_tiles

        # ==================================================================
        # Phase 4: MA block (token-chunked); x2 persists for GCN
        # ==================================================================
        x2pool = ctx.enter_context(tc.tile_pool(name="x2p", bufs=1))
        x2_fm = [x2pool.tile([P, nloc], F, tag=f"x2{kt}", name=f"x2_{kt}") for kt in range(2)]
        x_cat = [x0_fm, xg_fm]
        with tc.tile_pool(name="d4", bufs=1) as d4pool, \
             tc.tile_pool(name="ln4", bufs=1) as ln4pool, \
             tc.tile_pool(name="d4ps", bufs=3, space="PSUM") as d4ps, \
             tc.tile_pool(name="r4ps", bufs=1, space="PSUM") as r4ps:
            dn.update(pool=d4pool, ln=ln4pool, ps=d4ps, rowps=r4ps)
            for tch in range(TCH):
                tsl = slice(tch * CH_T, (tch + 1) * CH_T)
                xcs = [_SliceAP(t[:, tsl]) for t in x_cat]
                p_fm = mm_fm(xcs, CH_T, "ma1w_t", "ma1_b", AF.Lrelu, "pfm")
                p2_fm = mm_fm(p_fm, CH_T, "ma2w_t", "ma2_b", AF.Lrelu, "p2fm")
                v_fm = mm_fm(p2_fm, CH_T, "lvw_t", "lv_b", None, "vfm")
                a1_fm = mm_fm(v_fm, CH_T, "wvw_t", "wv_b", None, "a1fm")
                atto = mm_fm(a1_fm, CH_T, "outw_t", "out_b", None, "aofm")
                pl = layer_norm(atto, CH_T, "maln_g", "maln_b", invn256,
                                out_tag="plfm")
                for kt in range(2):
                    nc.scalar.activation(pl[kt][:], pl[kt][:], AF.Lrelu)
                    nc.vector.tensor_add(pl[kt][:], pl[kt][:], x_cat[kt][:, tsl])
                layer_norm(pl, CH_T, "fn1_g", "fn1_b", invn256,
                           out_tiles=[_SliceAP(x2_fm[kt][:, tsl])
                                      for kt in range(2)])

        # ==================================================================
        # Phase 5: GRU weight evolution + xw + AllGather
        # ==================================================================
        with tc.tile_pool(name="gru", bufs=1) as gp, \
             tc.tile_pool(name="grups", bufs=1, space="PSUM") as gps, \
             tc.tile_pool(name="xwps", bufs=3, space="PSUM") as xwps:
            brz_rep = replicate_row(wt["brz_row"][0], 2 * DIM2, "brzr", gp)
            bin_rep = replicate_row(wt["bin_row"][0], DIM2, "binr", gp)
            bhn_rep = replicate_row(wt["bhn_row"][0], DIM2, "bhnr", gp)
            rz, W_row = [], []
            for mt in range(2):
                ps = gps.tile([P, 2 * DIM2], F, tag="rzps")
                for kt in range(2):
                    nc.tensor.matmul(ps[:],
                                     lhsT=wt["egw_t"][kt][:, mt * P:(mt + 1) * P],
                                     rhs=wt["wrz_t"][kt][:],
                                     start=(kt == 0), stop=(kt == 1))
                t = gp.tile([P, 2 * DIM2], F, tag=f"rz{mt}")
                nc.vector.tensor_add(t[:], ps[:], brz_rep[:])
                nc.scalar.activation(t[:], t[:], AF.Sigmoid)
                rz.append(t)
            for mt in range(2):
                ps3 = gps.tile([P, DIM2], F, tag="gi3ps")
                for kt in range(2):
                    nc.tensor.matmul(ps3[:],
                                     lhsT=wt["egw_t"][kt][:, mt * P:(mt + 1) * P],
                                     rhs=wt["win_t"][kt][:],
                                     start=(kt == 0), stop=(kt == 1))
                gi3 = gp.tile([P, DIM2], F, tag=f"gi3{mt}")
                nc.vector.tensor_add(gi3[:], ps3[:], bin_rep[:])
                ps4 = gps.tile([P, DIM2], F, tag="gh3ps")
                for kt in range(2):
                    nc.tensor.matmul(ps4[:],
                                     lhsT=wt["egw_t"][kt][:, mt * P:(mt + 1) * P],
                                     rhs=wt["whn_t"][kt][:],
                                     start=(kt == 0), stop=(kt == 1))
                gh3 = gp.tile([P, DIM2], F, tag=f"gh3{mt}")
                nc.vector.tensor_add(gh3[:], ps4[:], bhn_rep[:])
                nc.vector.tensor_mul(gh3[:], gh3[:], rz[mt][:, :DIM2])
                nc.vector.tensor_add(gh3[:], gh3[:], gi3[:])
                nc.scalar.activation(gh3[:], gh3[:], AF.Tanh)
                wr = gp.tile([P, DIM2], F, tag=f"wrow{mt}")
                nc.vector.tensor_sub(wr[:], wt["egw_row"][mt][:], gh3[:])
                nc.vector.tensor_mul(wr[:], wr[:], rz[mt][:, DIM2:])
                nc.vector.tensor_add(wr[:], wr[:], gh3[:])
                W_row.append(wr)
            for blk in range(LBLK):
                sl = slice(blk * P, (blk + 1) * P)
                ps = xwps.tile([P, DIM2], F, tag="xwp")
                for kt in range(2):
                    nc.tensor.matmul(ps[:], lhsT=x2_fm[kt][:, sl],
                                     rhs=W_row[kt][:],
                                     start=(kt == 0), stop=(kt == 1))
                sb = gp.tile([P, DIM2], F, tag="xwsb")
                nc.vector.tensor_copy(sb[:], ps[:])
                nc.sync.dma_start(xw_loc[sl, :], sb[:])
            nc.gpsimd.collective_compute(
                "AllGather", OP.bypass,
                replica_groups=[list(range(n_cores))],
                ins=[xw_loc.opt()], outs=[xw_full.opt()])

        # ==================================================================
        # Phase 6: GCN edge loop -> gfm = lrelu(gcn) + x2 (per range)
        # ==================================================================
        gfmpool = ctx.enter_context(tc.tile_pool(name="gfmp", bufs=1))
        gfm = [gfmpool.tile([P, nloc], F, tag=f"gfm{kt}", name=f"gfm{kt}") for kt in range(2)]
        with tc.tile_pool(name="e6c", bufs=1) as e6c:
            idx_t6 = e6c.tile([P, 8 * COLS], I16, tag="idx6")
            nc.sync.dma_start(idx_t6[:], srcidx_d[:])
            nrm_t = e6c.tile([P, COLS], F, tag="nrm")
            nc.sync.dma_start(nrm_t[:], normv_d[:])
            with tc.tile_pool(name="e6", bufs=3) as e6, \
                 tc.tile_pool(name="e6a", bufs=2 * CC) as e6a, \
                 tc.tile_pool(name="e6post", bufs=2) as e6post, \
                 tc.tile_pool(name="gcnps", bufs=2, space="PSUM") as gcnps, \
                 tc.tile_pool(name="gtrps", bufs=2, space="PSUM") as gtrps:
                g_ps = None
                for ch in range(COLS // CC):
                    cols = [ch * CC + i for i in range(CC)]
                    xwg = e6.tile([P, CC, DIM2], F, tag="xwg")
                    nc.gpsimd.dma_gather(
                        xwg[:], xw_full[:],
                        idx_t6[:, ch * CC * 8:(ch + 1) * CC * 8],
                        CC * P, CC * P, DIM2, single_packet=(CC * P <= 1024))
                    nc.vector.tensor_mul(
                        xwg[:], xwg[:],
                        nrm_t[:, ch * CC:(ch + 1) * CC].unsqueeze(2)
                        .to_broadcast([P, CC, DIM2]))
                    for i, col in enumerate(cols):
                        r = col // CPR
                        first = col % CPR == 0
                        last = col % CPR == CPR - 1
                        a_t = e6a.tile([P, P], F, tag="Asb6")
                        nc.sync.dma_start(a_t[:], A_d[col, :, :])
                        if first:
                            g_ps = gcnps.tile([P, DIM2], F, tag="gcnp")
                        nc.tensor.matmul(g_ps[:], lhsT=a_t[:], rhs=xwg[:, i, :],
                                         start=first, stop=last)
                        if last:
                            gsb = e6post.tile([P, DIM2], F, tag="gsb")
                            nc.scalar.copy(gsb[:], g_ps[:])
                            osl = slice(r * P, (r + 1) * P)
                            for kt in range(2):
                                tr = gtrps.tile([P, P], F, tag="gtr6")
                                nc.tensor.transpose(
                                    tr[:], gsb[:, kt * P:(kt + 1) * P], ident[:])
                                nc.scalar.activation(gfm[kt][:, osl], tr[:],
                                                     AF.Lrelu)
                                nc.vector.tensor_add(gfm[kt][:, osl],
                                                     gfm[kt][:, osl],
                                                     x2_fm[kt][:, osl])

        # ==================================================================
        # Phase 7+8: x3 = LN(gfm); head; outputs (token-chunked)
        # ==================================================================
        with tc.tile_pool(name="d8", bufs=1) as d8pool, \
             tc.tile_pool(name="ln8", bufs=1) as ln8pool, \
             tc.tile_pool(name="p8", bufs=2) as p8, \
             tc.tile_pool(name="d8ps", bufs=2, space="PSUM") as d8ps, \
             tc.tile_pool(name="r8ps", bufs=1, space="PSUM") as r8ps, \
             tc.tile_pool(name="p8ps", bufs=2, space="PSUM") as p8ps:
            dn.update(pool=d8pool, ln=ln8pool, ps=d8ps, rowps=r8ps)
            for tch in range(TCH):
                tsl = slice(tch * CH_T, (tch + 1) * CH_T)
                gslc = [_SliceAP(gfm[kt][:, tsl]) for kt in range(2)]
                x3 = layer_norm(gslc, CH_T, "nev_g", "nev_b", invn256,
                                out_tag="x3fm")
                h1 = mm_fm(x3, CH_T, "fc1w_t", "fc1_b", AF.Lrelu, "h1t")
                h2 = mm_fm(h1, CH_T, "fc2w_t", "fc2_b", AF.Lrelu, "h2t")
                o_fm = layer_norm(h2, CH_T, "ng_g", "ng_b", invn128,
                                  out_tag="ofm")
                for gg in range(CH_T // P):
                    g = tch * (CH_T // P) + gg
                    ps = p8ps.tile([P, P], F, tag="otr")
                    nc.tensor.transpose(ps[:], o_fm[0][:, gg * P:(gg + 1) * P],
                                        ident[:])
                    osb = p8.tile([P, P], F, tag="osb")
                    nc.vector.tensor_copy(osb[:], ps[:])
                    nc.sync.dma_start(out_mat_d[g * P:(g + 1) * P, :], osb[:])
                lrow = p8.tile([1, CH_T], F, tag="lrow")
                for j in range(CH_T // 512):
                    sl = slice(j * 512, (j + 1) * 512)
                    ps = p8ps.tile([1, 512], F, tag="lps")
                    nc.tensor.matmul(ps[:], lhsT=wt["clsw_col"][0][:, :1],
                                     rhs=o_fm[0][:, sl], start=True, stop=True)
                    nc.scalar.activation(lrow[:, sl], ps[:], AF.Identity,
                                         bias=wt["cls_b"][0][:1, :1])
                nc.sync.dma_start(out_line_d[:, tsl], lrow[:])

    nc.compile()
    return nc


# --------------------------------------------------------------------------
# host entry point
# --------------------------------------------------------------------------

def _prepare(inputs, n_cores, nn, nloc):
    feature = np.asarray(inputs["feature"], np.float32)
    adj = np.asarray(inputs["adj"], np.int64)
    src = np.concatenate([adj[0], np.arange(nn, dtype=np.int64)])
    dst = np.concatenate([adj[1], np.arange(nn, dtype=np.int64)])
    CPR, per_core = pack_graph(src, dst, n_cores, nloc, nn)
    w = prep_weights(inputs)
    featT = np.ascontiguousarray(feature.T)
    in_maps = []
    for c in range(n_cores):
        m = dict(per_core[c])
        m["featT"] = featT
        m["featTloc"] = np.ascontiguousarray(featT[:, c * nloc:(c + 1) * nloc])
        for k in WEIGHT_SHAPES:
            m[k] = w[k]
        in_maps.append(m)
    return CPR, in_maps


def run(inputs, n_cores=NCORES, nn=N, CC=2, CH_T=1024, trace=False):
    nloc = nn // n_cores
    CPR, in_maps = _prepare(inputs, n_cores, nn, nloc)
    key = (n_cores, nn, nloc, CPR, CC, CH_T)
    if key not in _BUILD_CACHE:
        _BUILD_CACHE[key] = build_bass(*key)
    nc = _BUILD_CACHE[key]
    out = run_bass_kernel_spmd(nc, in_maps, list(range(n_cores)), trace=trace)
    res = out.results
    out_line = np.concatenate([r["out_line"][0] for r in res])
    out_mat = np.concatenate([r["out_mat"] for r in res], axis=0)
    return (out_line, out_mat), out


def kernel(**inputs):
    (out_line, out_mat), _ = run(inputs)
    return out_line, out_mat


# --------------------------------------------------------------------------
# benchmarking (repeated execution with device-resident inputs)
# --------------------------------------------------------------------------

def bench(inputs, n_cores=NCORES, nn=N, CC=2, CH_T=1024, iters=10):
    """Time the NEFF execution via the PJRT path, amortizing dispatch.

    Returns (per_call_s_chained, per_call_s_sync, results_first_call).
    """
    import time
    import jax
    from jax.sharding import Mesh, PartitionSpec
    from jax.experimental.shard_map import shard_map
    from concourse import bass2jax, mybir as mb

    nloc = nn // n_cores
    CPR, in_maps = _prepare(inputs, n_cores, nn, nloc)
    key = (n_cores, nn, nloc, CPR, CC, CH_T)
    if key not in _BUILD_CACHE:
        _BUILD_CACHE[key] = build_bass(*key)
    nc = _BUILD_CACHE[key]
    bass2jax.install_neuronx_cc_hook()

    partition_name = nc.partition_id_tensor.name if nc.partition_id_tensor else None
    in_names, out_names, out_avals, zero_outs = [], [], [], []
    for alloc in nc.m.functions[0].allocations:
        if not isinstance(alloc, mb.MemoryLocationSet):
            continue
        name = alloc.memorylocations[0].name
        if alloc.kind == "ExternalInput":
            if name != partition_name:
                in_names.append(name)
        elif alloc.kind == "ExternalOutput":
            out_names.append(name)
            shape = tuple(alloc.tensor_shape)
            dtype = mb.dt.np(alloc.dtype)
            out_avals.append(jax.core.ShapedArray(shape, dtype))
            zero_outs.append(np.zeros(shape, dtype))
    n_params = len(in_names)
    all_in_names = in_names + out_names + ([partition_name] if partition_name else [])

    def _body(*args):
        operands = list(args)
        if partition_name is not None:
            operands.append(bass2jax.partition_id_tensor())
        outs = bass2jax._bass_exec_p.bind(
            *operands,
            out_avals=tuple(out_avals),
            in_names=tuple(all_in_names),
            out_names=tuple(out_names),
            lowering_input_output_aliases=(),
            sim_require_finite=True,
            sim_require_nnan=True,
            nc=nc,
        )
        return tuple(outs)

    devices = jax.devices()[:n_cores]
    mesh = Mesh(np.asarray(devices), ("core",))
    nin = n_params + len(out_names)
    fn = jax.jit(
        shard_map(_body, mesh=mesh,
                  in_specs=(PartitionSpec("core"),) * nin,
                  out_specs=(PartitionSpec("core"),) * len(out_names),
                  check_rep=False),
        keep_unused=True,
    )
    concat_in = [np.concatenate([np.asarray(in_maps[c][n]) for c in range(n_cores)], 0)
                 for n in in_names]
    concat_zero = [np.zeros((n_cores * z.shape[0], *z.shape[1:]), z.dtype)
                   for z in zero_outs]
    sh = jax.sharding.NamedSharding(mesh, PartitionSpec("core"))
    dev_in = [jax.device_put(a, sh) for a in concat_in + concat_zero]
    out0 = jax.block_until_ready(fn(*dev_in))

    # sync per-call timing
    t0 = time.time()
    for _ in range(iters):
        jax.block_until_ready(fn(*dev_in))
    t_sync = (time.time() - t0) / iters
    # chained timing (amortizes dispatch)
    t0 = time.time()
    rs = [fn(*dev_in) for _ in range(iters)]
    jax.block_until_ready(rs)
    t_chain = (time.time() - t0) / iters
    return t_chain, t_sync, (out0, out_names)
